# revision 24
# baseline (speedup 1.0000x reference)
"""TRN2 Bass/Tile kernel: graph neural ODE integrated with RK4.

Computes pred_y[t] for t=0..19 where
    dx/dt = f(x) = tanh((edge @ x) @ W1 + x @ W2 + b)
via 19 RK4 steps from x0 = node, data-parallel over the batch axis:
16 batches sharded 2-per-core across 8 NeuronCores (SPMD, no collectives).

Layout strategy (per core, per batch):
  - State lives TRANSPOSED in SBUF: xT[d, i]  (feature on partitions,
    512 nodes on the free axis).
  - v-stage:  v = y @ W1 in natural [node, feat] layout via
        matmul(lhsT=yT[:, c*128:(c+1)*128], rhs=[W1|W2])  -> psum[j, 256]
    (the widened [W1|W2] moving operand keeps the fp32r fast path,
    which needs a moving free dim >= 256; the W2 half is discarded).
  - z-stage:  zT[e, i] = (edge @ v)^T + (y @ W2)^T accumulated in one
    PSUM bank.
  - Z/V-LINEARITY: z() and v() are linear, so the RK4 intermediate
    states never materialize:  v(x + c*k) = V1 + c*v(k), and the
    W2-term splits as (x + c*k) @ W2 = x@W2 + c*(k@W2).  Eval 1 keeps
    its v in SBUF (V1); evals 2-4 run the v-stage on k_{e-1} with
    pre-scaled weights and rebuild the full v_e = V1 + c*v(k) in the
    PSUM->SBUF transfer (DVE add).  The z PSUM group is seeded by an
    x@W2 matmul (re-run per eval — same PE cost as an identity-seed
    matmul, but no Z1 SBUF snapshot is needed).
  - tanh on ScalarE straight out of PSUM with per-partition bias b.
  - RK4 combine:  x_new = x + dt/6 (k1 + k4) + dt/3 (k2 + k3), computed as
        u = x + dt/6 k1   (DVE STT, after k1)
        m = k2 + k3       (Pool tensor_tensor add — GPSIMD supports plain
                           Add but not fused scalar_tensor_tensor)
        u2 = u + dt/3 m   (DVE STT, after k3)
        x_new = u2 + dt/6 k4  (DVE STT, the only combine op on the k4 tail)
    The state master is the f32r x_new itself (no separate f32 shadow);
    the per-step f32r rounding error (~1e-4 relative per step) is far
    inside the 2e-2 gate.
  - edge is consumed transposed (edgeT[j, i]); the host pre-transposes it
    (free), and the per-step outputs are written transposed [d, i] and
    un-transposed on the host (also free).

Engine budget per step (both batches, [128,512]-sized elementwise ops):
  ACT: 8 tanh + 2 eval-1 v-copies; DVE: 6 v-adds + 6 combine STT;
  Pool: 2 m-adds.  PE: ~31.7k cycles of fp32r matmul.

All matmuls run in float32r (fp32 rounded to 11 explicit mantissa bits;
the PE runs 1 cycle/row for moving dims >= 256).  Values feeding matmuls
are produced as float32r (DMA of host-pre-rounded data, ACT tanh/copy
outputs, DVE outputs), which is what the walrus verifier requires.
"""

import numpy as np

import concourse.tile as tile
from concourse import bacc, mybir
from concourse import bass_utils

B, N, D, T = 16, 512, 128, 20
NCORES = 8
BPC = B // NCORES  # batches per core

F32 = mybir.dt.float32
F32R = mybir.dt.float32r
ALU = mybir.AluOpType
ACTF = mybir.ActivationFunctionType
SKEW = 4  # batch-1 emission lag, in eval slots (software pipelining)


def build_program(dts, repeat=1, unroll=1):
    """Build the SPMD Bass program (identical on all cores).

    repeat > 1 re-runs the whole integration from x0 that many times via a
    hardware For_i loop; unroll > 1 additionally python-unrolls that many
    passes inside the loop body (timing runs only; the output stays that of
    the final pass).  Comparing unroll=u vs unroll=1 at the same repeat
    cancels both the per-dispatch overhead and the For_i loop-boundary
    overhead, isolating the true straight-line per-pass time.
    """
    nc = bacc.Bacc(
        "TRN2",
        target_bir_lowering=False,
        debug=False,
        num_devices=NCORES,
    )
    dt_vals = sorted({float(d) for d in dts})
    nodeT_in = nc.dram_tensor("nodeT", [BPC, D, N], F32R, kind="ExternalInput").ap()
    edgeT_in = nc.dram_tensor("edgeT", [BPC, N, N], F32R, kind="ExternalInput").ap()
    # per distinct dt: [W1|W2], c/2*[W1|W2], c*[W1|W2] are slices of wcats
    wcats_in = nc.dram_tensor(
        "wcats", [1 + 2 * len(dt_vals), D, 2 * D], F32R, kind="ExternalInput"
    ).ap()
    w2s_in = nc.dram_tensor(
        "w2s", [1 + 2 * len(dt_vals), D, D], F32R, kind="ExternalInput"
    ).ap()
    b_in = nc.dram_tensor("bvec", [D, 1], F32, kind="ExternalInput").ap()
    out_t = nc.dram_tensor("out", [T - 1, BPC, D, N], F32, kind="ExternalOutput").ap()

    with tile.TileContext(nc) as tc:
        _emit(
            tc, nodeT_in, edgeT_in, wcats_in, w2s_in,
            b_in, out_t, dts, dt_vals, repeat, unroll,
        )
    nc.compile()
    return nc


def _emit(tc, nodeT_in, edgeT_in, wcats_in, w2s_in,
          b_in, out_t, dts, dt_vals, repeat, unroll=1):
    from contextlib import ExitStack

    nc = tc.nc
    nw = 1 + 2 * len(dt_vals)
    with ExitStack() as ctx:
        const = ctx.enter_context(tc.tile_pool(name="const", bufs=1))
        state = ctx.enter_context(tc.tile_pool(name="state", bufs=2))
        kpool = ctx.enter_context(tc.tile_pool(name="k", bufs=2))
        v1pool = ctx.enter_context(tc.tile_pool(name="v1", bufs=2))
        vepool = ctx.enter_context(tc.tile_pool(name="ve", bufs=2))
        tmp = ctx.enter_context(tc.tile_pool(name="tmp", bufs=2))
        pv = ctx.enter_context(tc.tile_pool(name="pv", bufs=1, space="PSUM"))
        pz = ctx.enter_context(tc.tile_pool(name="pz", bufs=2, space="PSUM"))

        # DMA order = queue order at startup; order loads by first use.
        # Eval 1 needs only wcat slice 0, w2 slice 0, x0 and bias; the
        # dt-scaled weight slices are first touched by eval 2 (~4us in),
        # so they queue after x0.
        # eval-1's weight slices live in their own tiles: tile-granular
        # dependency tracking would otherwise make the first matmuls wait
        # for the later-queued scaled slices written into the same tile
        wcat0 = const.tile([D, 2 * D], F32R, tag="wcat0")
        w2s0 = const.tile([D, D], F32R, tag="w2s0")
        wcats = const.tile([D, (nw - 1) * 2 * D], F32R, tag="wcats")
        w2s = const.tile([D, (nw - 1) * D], F32R, tag="w2s")
        nc.sync.dma_start(wcat0[:], wcats_in[0])
        nc.sync.dma_start(w2s0[:], w2s_in[0])

        def wcat_slice(idx):
            if idx == 0:
                return wcat0[:]
            return wcats[:, (idx - 1) * 2 * D : idx * 2 * D]

        def w2_slice(idx):
            if idx == 0:
                return w2s0[:]
            return w2s[:, (idx - 1) * D : idx * D]

        def load_x0():
            xs = []
            for bb in range(BPC):
                x0 = state.tile([D, N], F32R, tag=f"x{bb}")
                nc.sync.dma_start(x0[:], nodeT_in[bb])
                xs.append(x0)
            return xs

        x0_pre = load_x0() if repeat == 1 and unroll == 1 else None

        bias = const.tile([D, 1], F32, tag="bias")
        nc.sync.dma_start(bias[:], b_in)
        for w in range(1, nw):
            nc.sync.dma_start(wcats[:, (w - 1) * 2 * D : w * 2 * D], wcats_in[w])
            nc.sync.dma_start(w2s[:, (w - 1) * D : w * D], w2s_in[w])

        edge_sb = [
            const.tile([128, 4 * N], F32R, tag=f"edge{bb}", name=f"edge{bb}")
            for bb in range(BPC)
        ]
        for c in range(4):
            for bb in range(BPC):
                # spread the 2MB of edge loads over both HWDGE queues
                # (b0 on the otherwise-empty ACT queue, b1 on SP; shifting
                # b1 chunks onto ACT measured worse in the timeline model)
                eng = nc.scalar if (c * BPC + bb) % 2 == 0 else nc.sync
                eng.dma_start(
                    edge_sb[bb][:, c * N : (c + 1) * N],
                    edgeT_in[bb, c * 128 : (c + 1) * 128, :],
                )

        def emit_vstage(bb, y, widx):
            """psum v-tile: [y@(c W1) | y@(c W2)] per 128-node chunk."""
            pvt = pv.tile([128, 4 * 256], F32, tag=f"pv{bb}")
            for c in range(4):
                nc.tensor.matmul(
                    pvt[:, c * 256 : (c + 1) * 256],
                    lhsT=y[:, c * 128 : (c + 1) * 128],
                    rhs=wcat_slice(widx),
                    start=True,
                    stop=True,
                )
            return pvt

        def emit_vcopy(bb, pvt):
            """eval 1: V1 = x@W1, plain PSUM->SBUF copy on ACT (pinned)."""
            vt = v1pool.tile([128, N], F32R, tag=f"v1_{bb}", name=f"v1_{bb}")
            dst = vt[:].rearrange("p (c e) -> p c e", c=4)
            src = pvt[:].rearrange("p (c w) -> p c w", c=4)[:, :, 0:128]
            nc.scalar.activation(dst, src, ACTF.Copy)
            return vt

        def emit_vadd(bb, pvt, v1t):
            """evals 2-4: v_e = V1 + c*(k@W1) in the PSUM->SBUF move (DVE)."""
            vt = vepool.tile([128, N], F32R, tag=f"ve_{bb}", name=f"ve_{bb}")
            dst = vt[:].rearrange("p (c e) -> p c e", c=4)
            src = pvt[:].rearrange("p (c w) -> p c w", c=4)[:, :, 0:128]
            v1s = v1t[:].rearrange("p (c e) -> p c e", c=4)
            nc.vector.scalar_tensor_tensor(dst, src, 1.0, v1s, ALU.mult, ALU.add)
            return vt

        def emit_zstage(bb, x, y, widx):
            """psum z group, part 1: x@W2 seed (start=True) and, for evals
            2-4, the c*(k@W2) term — these depend only on x/k, so they can
            run during the v-copy/add."""
            pzt = pz.tile([128, N], F32, tag=f"pz{bb}")
            nc.tensor.matmul(
                pzt[:], lhsT=w2_slice(0), rhs=x[:], start=True, stop=False
            )
            if widx != 0:
                nc.tensor.matmul(
                    pzt[:], lhsT=w2_slice(widx), rhs=y[:], start=False, stop=False
                )
            return pzt

        def emit_zstage_agg(bb, vt, pzt):
            for c in range(4):
                nc.tensor.matmul(
                    pzt[:],
                    lhsT=vt[:, c * 128 : (c + 1) * 128],
                    rhs=edge_sb[bb][:, c * N : (c + 1) * N],
                    start=False,
                    stop=(c == 3),
                )
            return pzt

        loop_ctx = tc.For_i(0, repeat, 1) if repeat > 1 else None
        if loop_ctx is not None:
            ctx.enter_context(loop_ctx)
        def make_batch_emitter(bb, x0):
            """Closure emitting one (t, e) eval for batch bb per call."""
            st = {"x": x0, "ks": [None] * 4, "u": None, "u2": None, "v1": None}

            def emit_eval(t, e):
                dt = float(dts[t])
                di = dt_vals.index(dt)
                w_half = 1 + 2 * di      # (dt/2) * [W1|W2]
                w_full_dt = 2 + 2 * di   # dt * [W1|W2]
                widx = (0, w_half, w_half, w_full_dt)[e]
                y = st["x"] if e == 0 else st["ks"][e - 1]
                pvt = emit_vstage(bb, y, widx)
                if e == 0:
                    vt = emit_vcopy(bb, pvt)
                    st["v1"] = vt
                else:
                    vt = emit_vadd(bb, pvt, st["v1"])
                pzt = emit_zstage(bb, st["x"], y, widx)
                emit_zstage_agg(bb, vt, pzt)
                k = kpool.tile([D, N], F32R, tag=f"k{e}_{bb}", name=f"k{e}_{bb}")
                nc.scalar.activation(k[:], pzt[:], ACTF.Tanh, bias=bias[:])
                st["ks"][e] = k
                # RK4 combine, incremental and mostly off the k4 tail:
                #   e0: u = x + dt/6 k1 (DVE)   e2: m = k2+k3 (Pool);
                #                               u2 = u + dt/3 m (DVE)
                #   e3: x_new = u2 + dt/6 k4 (DVE, f32r master)
                if e == 0:
                    u = tmp.tile([D, N], F32, tag=f"u{bb}")
                    nc.vector.scalar_tensor_tensor(
                        u[:], k[:], dt / 6.0, st["x"][:], ALU.mult, ALU.add
                    )
                    st["u"] = u
                elif e == 2:
                    m = tmp.tile([D, N], F32, tag=f"m{bb}")
                    nc.gpsimd.tensor_tensor(
                        m[:], st["ks"][1][:], st["ks"][2][:], ALU.add
                    )
                    u2 = tmp.tile([D, N], F32, tag=f"u2{bb}")
                    nc.vector.scalar_tensor_tensor(
                        u2[:], m[:], dt / 3.0, st["u"][:], ALU.mult, ALU.add
                    )
                    st["u2"] = u2
                elif e == 3:
                    x_new = state.tile([D, N], F32R, tag=f"x{bb}")
                    nc.vector.scalar_tensor_tensor(
                        x_new[:], k[:], dt / 6.0, st["u2"][:], ALU.mult, ALU.add
                    )
                    nc.sync.dma_start(out_t[t, bb], x_new[:].bitcast(F32))
                    st["x"] = x_new

            return emit_eval

        for rep in range(unroll):
            x_cur = x0_pre if x0_pre is not None else load_x0()
            emitters = [make_batch_emitter(bb, x_cur[bb]) for bb in range(BPC)]
            slots = [(t, e) for t in range(T - 1) for e in range(4)]
            # Software-pipeline the two independent batch chains with a
            # SKEW-eval emission offset: each engine's static instruction
            # order then alternates between ops that are a full eval apart
            # in dependency distance, so a stalled spine op of one batch
            # never head-of-line-blocks a ready op of the other.
            n = len(slots)
            for s in range(n + SKEW):
                if s < n:
                    emitters[0](*slots[s])
                if SKEW <= s:
                    emitters[1](*slots[s - SKEW])


def round_f32r(x):
    """Round fp32 values to the fp32r subset (11 explicit mantissa bits,
    low 12 bits zero) with round-to-nearest-even — matches what the PE
    consumes in fp32r mode, so host-side rounding keeps hardware exact."""
    u = np.ascontiguousarray(x, dtype=np.float32).view(np.uint32)
    u = (u + 0x7FF + ((u >> 12) & 1)) & np.uint32(0xFFFFF000)
    return u.view(np.float32)


def make_in_maps(node, edge, time_steps, W1, W2, b):
    dts = np.asarray(time_steps, np.float32)
    dts = dts[1:] - dts[:-1]
    dt_vals = sorted({float(d) for d in dts})
    wcat = np.concatenate([W1, W2], axis=1).astype(np.float32)
    wcats = [wcat]
    w2s = [W2.astype(np.float32)]
    for dv in dt_vals:
        wcats.append(wcat * (dv / 2))
        wcats.append(wcat * dv)
        w2s.append(W2 * (dv / 2))
        w2s.append(W2 * dv)
    wcats = round_f32r(np.stack(wcats))
    w2s = round_f32r(np.stack(w2s))
    bc = np.ascontiguousarray(np.reshape(b, (D, 1)), dtype=np.float32)
    in_maps = []
    for core in range(NCORES):
        sl = slice(core * BPC, (core + 1) * BPC)
        in_maps.append(
            {
                "nodeT": round_f32r(node[sl].transpose(0, 2, 1)),
                "edgeT": round_f32r(edge[sl].transpose(0, 2, 1)),
                "wcats": wcats,
                "w2s": w2s,
                "bvec": bc,
            }
        )
    return in_maps


LAST_RESULT = None


def kernel(node, edge, time_steps, W1, W2, b, trace=False):
    node = np.asarray(node, dtype=np.float32)
    edge = np.asarray(edge, dtype=np.float32)
    time_steps = np.asarray(time_steps, dtype=np.float32)
    W1 = np.asarray(W1, dtype=np.float32)
    W2 = np.asarray(W2, dtype=np.float32)
    b = np.asarray(b, dtype=np.float32)

    dts = time_steps[1:] - time_steps[:-1]
    nc = build_program(dts)
    in_maps = make_in_maps(node, edge, time_steps, W1, W2, b)
    res = bass_utils.run_bass_kernel_spmd(
        nc, in_maps, core_ids=list(range(NCORES)), trace=trace
    )
    global LAST_RESULT
    LAST_RESULT = res
    outs = [res.results[c]["out"] for c in range(NCORES)]  # [T-1, BPC, D, N]
    full = np.concatenate(outs, axis=1)  # [T-1, B, D, N]
    pred = np.empty((T, B, N, D), dtype=np.float32)
    pred[0] = node
    pred[1:] = full.transpose(0, 1, 3, 2)
    return pred


# revision 25
# speedup vs baseline: 1.1173x; 1.1173x over previous
"""TRN2 Bass/Tile kernel: graph neural ODE integrated with RK4.

Computes pred_y[t] for t=0..19 where
    dx/dt = f(x) = tanh((edge @ x) @ W1 + x @ W2 + b)
via 19 RK4 steps from x0 = node, data-parallel over the batch axis:
16 batches sharded 2-per-core across 8 NeuronCores (SPMD, no collectives).

Layout strategy (per core, per batch):
  - State lives TRANSPOSED in SBUF: xT[d, i]  (feature on partitions,
    512 nodes on the free axis).
  - v-stage:  v = y @ W1 in natural [node, feat] layout via
        matmul(lhsT=yT[:, c*128:(c+1)*128], rhs=[W1|W2])  -> psum[j, 256]
    (the widened [W1|W2] moving operand keeps the fp32r fast path,
    which needs a moving free dim >= 256; the W2 half is discarded).
  - z-stage:  zT[e, i] = (edge @ v)^T + (y @ W2)^T accumulated in one
    PSUM bank.
  - Z/V-LINEARITY: z() and v() are linear, so the RK4 intermediate
    states never materialize:  v(x + c*k) = V1 + c*v(k), and the
    W2-term splits as (x + c*k) @ W2 = x@W2 + c*(k@W2).  Eval 1 keeps
    its v in SBUF (V1); evals 2-4 run the v-stage on k_{e-1} with
    pre-scaled weights and rebuild the full v_e = V1 + c*v(k) in the
    PSUM->SBUF transfer (DVE add).  The z PSUM group is seeded by an
    x@W2 matmul (re-run per eval — same PE cost as an identity-seed
    matmul, but no Z1 SBUF snapshot is needed).
  - tanh on ScalarE straight out of PSUM with per-partition bias b.
  - RK4 combine:  x_new = x + dt/6 (k1 + k4) + dt/3 (k2 + k3), computed as
        u = x + dt/6 k1   (DVE STT, after k1)
        m = k2 + k3       (Pool tensor_tensor add — GPSIMD supports plain
                           Add but not fused scalar_tensor_tensor)
        u2 = u + dt/3 m   (DVE STT, after k3)
        x_new = u2 + dt/6 k4  (DVE STT, the only combine op on the k4 tail)
    The state master is the f32r x_new itself (no separate f32 shadow);
    the per-step f32r rounding error (~1e-4 relative per step) is far
    inside the 2e-2 gate.
  - edge is consumed transposed (edgeT[j, i]); the host pre-transposes it
    (free), and the per-step outputs are written transposed [d, i] and
    un-transposed on the host (also free).

Engine budget per step (both batches, [128,512]-sized elementwise ops):
  ACT: 8 tanh + 2 eval-1 v-copies; DVE: 6 v-adds + 6 combine STT;
  Pool: 2 m-adds.  PE: ~31.7k cycles of fp32r matmul.

All matmuls run in float32r (fp32 rounded to 11 explicit mantissa bits;
the PE runs 1 cycle/row for moving dims >= 256).  Values feeding matmuls
are produced as float32r (DMA of host-pre-rounded data, ACT tanh/copy
outputs, DVE outputs), which is what the walrus verifier requires.
"""

import numpy as np

import concourse.tile as tile
from concourse import bacc, mybir
from concourse import bass_utils

B, N, D, T = 16, 512, 128, 20
NCORES = 8
BPC = B // NCORES  # batches per core

F32 = mybir.dt.float32
F32R = mybir.dt.float32r
ALU = mybir.AluOpType
ACTF = mybir.ActivationFunctionType
SKEW = 0  # batch-1 emission lag, in eval slots (software pipelining)


def build_program(dts, repeat=1, unroll=1):
    """Build the SPMD Bass program (identical on all cores).

    repeat > 1 re-runs the whole integration from x0 that many times via a
    hardware For_i loop; unroll > 1 additionally python-unrolls that many
    passes inside the loop body (timing runs only; the output stays that of
    the final pass).  Comparing unroll=u vs unroll=1 at the same repeat
    cancels both the per-dispatch overhead and the For_i loop-boundary
    overhead, isolating the true straight-line per-pass time.
    """
    nc = bacc.Bacc(
        "TRN2",
        target_bir_lowering=False,
        debug=False,
        num_devices=NCORES,
    )
    dt_vals = sorted({float(d) for d in dts})
    nodeT_in = nc.dram_tensor("nodeT", [BPC, D, N], F32R, kind="ExternalInput").ap()
    edgeT_in = nc.dram_tensor("edgeT", [BPC, N, N], F32R, kind="ExternalInput").ap()
    # per distinct dt: [W1|W2], c/2*[W1|W2], c*[W1|W2] are slices of wcats
    wcats_in = nc.dram_tensor(
        "wcats", [1 + 2 * len(dt_vals), D, 2 * D], F32R, kind="ExternalInput"
    ).ap()
    w2s_in = nc.dram_tensor(
        "w2s", [1 + 2 * len(dt_vals), D, D], F32R, kind="ExternalInput"
    ).ap()
    b_in = nc.dram_tensor("bvec", [D, 1], F32, kind="ExternalInput").ap()
    out_t = nc.dram_tensor("out", [T - 1, BPC, D, N], F32, kind="ExternalOutput").ap()

    with tile.TileContext(nc) as tc:
        _emit(
            tc, nodeT_in, edgeT_in, wcats_in, w2s_in,
            b_in, out_t, dts, dt_vals, repeat, unroll,
        )
    nc.compile()
    return nc


def _emit(tc, nodeT_in, edgeT_in, wcats_in, w2s_in,
          b_in, out_t, dts, dt_vals, repeat, unroll=1):
    from contextlib import ExitStack

    nc = tc.nc
    nw = 1 + 2 * len(dt_vals)
    with ExitStack() as ctx:
        const = ctx.enter_context(tc.tile_pool(name="const", bufs=1))
        state = ctx.enter_context(tc.tile_pool(name="state", bufs=2))
        kpool = ctx.enter_context(tc.tile_pool(name="k", bufs=2))
        v1pool = ctx.enter_context(tc.tile_pool(name="v1", bufs=2))
        vepool = ctx.enter_context(tc.tile_pool(name="ve", bufs=2))
        tmp = ctx.enter_context(tc.tile_pool(name="tmp", bufs=2))
        pv = ctx.enter_context(tc.tile_pool(name="pv", bufs=1, space="PSUM"))
        pz = ctx.enter_context(tc.tile_pool(name="pz", bufs=2, space="PSUM"))

        # DMA order = queue order at startup; order loads by first use.
        # Eval 1 needs only wcat slice 0, w2 slice 0, x0 and bias; the
        # dt-scaled weight slices are first touched by eval 2 (~4us in),
        # so they queue after x0.
        # eval-1's weight slices live in their own tiles: tile-granular
        # dependency tracking would otherwise make the first matmuls wait
        # for the later-queued scaled slices written into the same tile
        wcat0 = const.tile([D, 2 * D], F32R, tag="wcat0")
        w2s0 = const.tile([D, D], F32R, tag="w2s0")
        wcats = const.tile([D, (nw - 1) * 2 * D], F32R, tag="wcats")
        w2s = const.tile([D, (nw - 1) * D], F32R, tag="w2s")
        nc.sync.dma_start(wcat0[:], wcats_in[0])
        nc.sync.dma_start(w2s0[:], w2s_in[0])

        def wcat_slice(idx):
            if idx == 0:
                return wcat0[:]
            return wcats[:, (idx - 1) * 2 * D : idx * 2 * D]

        def w2_slice(idx):
            if idx == 0:
                return w2s0[:]
            return w2s[:, (idx - 1) * D : idx * D]

        def load_x0():
            xs = []
            for bb in range(BPC):
                x0 = state.tile([D, N], F32R, tag=f"x{bb}")
                nc.sync.dma_start(x0[:], nodeT_in[bb])
                xs.append(x0)
            return xs

        x0_pre = load_x0() if repeat == 1 and unroll == 1 else None

        bias = const.tile([D, 1], F32, tag="bias")
        nc.sync.dma_start(bias[:], b_in)
        for w in range(1, nw):
            nc.sync.dma_start(wcats[:, (w - 1) * 2 * D : w * 2 * D], wcats_in[w])
            nc.sync.dma_start(w2s[:, (w - 1) * D : w * D], w2s_in[w])

        edge_sb = [
            const.tile([128, 4 * N], F32R, tag=f"edge{bb}", name=f"edge{bb}")
            for bb in range(BPC)
        ]
        for c in range(4):
            for bb in range(BPC):
                # spread the 2MB of edge loads over both HWDGE queues
                # (b0 on the otherwise-empty ACT queue, b1 on SP; shifting
                # b1 chunks onto ACT measured worse in the timeline model)
                eng = nc.scalar if (c * BPC + bb) % 2 == 0 else nc.sync
                eng.dma_start(
                    edge_sb[bb][:, c * N : (c + 1) * N],
                    edgeT_in[bb, c * 128 : (c + 1) * 128, :],
                )

        def emit_vstage(bb, y, widx):
            """psum v-tile: [y@(c W1) | y@(c W2)] per 128-node chunk."""
            pvt = pv.tile([128, 4 * 256], F32, tag=f"pv{bb}")
            for c in range(4):
                nc.tensor.matmul(
                    pvt[:, c * 256 : (c + 1) * 256],
                    lhsT=y[:, c * 128 : (c + 1) * 128],
                    rhs=wcat_slice(widx),
                    start=True,
                    stop=True,
                )
            return pvt

        def emit_vcopy(bb, pvt):
            """eval 1: V1 = x@W1, plain PSUM->SBUF copy on ACT (pinned)."""
            vt = v1pool.tile([128, N], F32R, tag=f"v1_{bb}", name=f"v1_{bb}")
            dst = vt[:].rearrange("p (c e) -> p c e", c=4)
            src = pvt[:].rearrange("p (c w) -> p c w", c=4)[:, :, 0:128]
            nc.scalar.activation(dst, src, ACTF.Copy)
            return vt

        def emit_vadd(bb, pvt, v1t):
            """evals 2-4: v_e = V1 + c*(k@W1) in the PSUM->SBUF move (DVE)."""
            vt = vepool.tile([128, N], F32R, tag=f"ve_{bb}", name=f"ve_{bb}")
            dst = vt[:].rearrange("p (c e) -> p c e", c=4)
            src = pvt[:].rearrange("p (c w) -> p c w", c=4)[:, :, 0:128]
            v1s = v1t[:].rearrange("p (c e) -> p c e", c=4)
            nc.vector.scalar_tensor_tensor(dst, src, 1.0, v1s, ALU.mult, ALU.add)
            return vt

        def emit_zstage(bb, x, y, widx):
            """psum z group, part 1: x@W2 seed (start=True) and, for evals
            2-4, the c*(k@W2) term — these depend only on x/k, so they can
            run during the v-copy/add."""
            pzt = pz.tile([128, N], F32, tag=f"pz{bb}")
            nc.tensor.matmul(
                pzt[:], lhsT=w2_slice(0), rhs=x[:], start=True, stop=False
            )
            if widx != 0:
                nc.tensor.matmul(
                    pzt[:], lhsT=w2_slice(widx), rhs=y[:], start=False, stop=False
                )
            return pzt

        def emit_zstage_agg(bb, vt, pzt):
            for c in range(4):
                nc.tensor.matmul(
                    pzt[:],
                    lhsT=vt[:, c * 128 : (c + 1) * 128],
                    rhs=edge_sb[bb][:, c * N : (c + 1) * N],
                    start=False,
                    stop=(c == 3),
                )
            return pzt

        loop_ctx = tc.For_i(0, repeat, 1) if repeat > 1 else None
        if loop_ctx is not None:
            ctx.enter_context(loop_ctx)
        def make_batch_emitter(bb, x0):
            """Closure emitting one (t, e) eval for batch bb per call."""
            st = {"x": x0, "ks": [None] * 4, "u": None, "u2": None, "v1": None}

            def emit_eval(t, e):
                dt = float(dts[t])
                di = dt_vals.index(dt)
                w_half = 1 + 2 * di      # (dt/2) * [W1|W2]
                w_full_dt = 2 + 2 * di   # dt * [W1|W2]
                widx = (0, w_half, w_half, w_full_dt)[e]
                y = st["x"] if e == 0 else st["ks"][e - 1]
                pvt = emit_vstage(bb, y, widx)
                pzt = emit_zstage(bb, st["x"], y, widx)
                if e == 0:
                    vt = emit_vcopy(bb, pvt)
                    st["v1"] = vt
                else:
                    vt = emit_vadd(bb, pvt, st["v1"])
                emit_zstage_agg(bb, vt, pzt)
                k = kpool.tile([D, N], F32R, tag=f"k{e}_{bb}", name=f"k{e}_{bb}")
                nc.scalar.activation(k[:], pzt[:], ACTF.Tanh, bias=bias[:])
                st["ks"][e] = k
                # RK4 combine, incremental and mostly off the k4 tail:
                #   e0: u = x + dt/6 k1 (DVE)   e2: m = k2+k3 (Pool);
                #                               u2 = u + dt/3 m (DVE)
                #   e3: x_new = u2 + dt/6 k4 (DVE, f32r master)
                if e == 0:
                    u = tmp.tile([D, N], F32, tag=f"u{bb}")
                    nc.vector.scalar_tensor_tensor(
                        u[:], k[:], dt / 6.0, st["x"][:], ALU.mult, ALU.add
                    )
                    st["u"] = u
                elif e == 2:
                    m = tmp.tile([D, N], F32, tag=f"m{bb}")
                    nc.gpsimd.tensor_tensor(
                        m[:], st["ks"][1][:], st["ks"][2][:], ALU.add
                    )
                    u2 = tmp.tile([D, N], F32, tag=f"u2{bb}")
                    nc.vector.scalar_tensor_tensor(
                        u2[:], m[:], dt / 3.0, st["u"][:], ALU.mult, ALU.add
                    )
                    st["u2"] = u2
                elif e == 3:
                    x_new = state.tile([D, N], F32R, tag=f"x{bb}")
                    nc.vector.scalar_tensor_tensor(
                        x_new[:], k[:], dt / 6.0, st["u2"][:], ALU.mult, ALU.add
                    )
                    nc.sync.dma_start(out_t[t, bb], x_new[:].bitcast(F32))
                    st["x"] = x_new

            return emit_eval

        for rep in range(unroll):
            x_cur = x0_pre if x0_pre is not None else load_x0()
            emitters = [make_batch_emitter(bb, x_cur[bb]) for bb in range(BPC)]
            slots = [(t, e) for t in range(T - 1) for e in range(4)]
            # Software-pipeline the two independent batch chains with a
            # SKEW-eval emission offset: each engine's static instruction
            # order then alternates between ops that are a full eval apart
            # in dependency distance, so a stalled spine op of one batch
            # never head-of-line-blocks a ready op of the other.
            n = len(slots)
            for s in range(n + SKEW):
                if s < n:
                    emitters[0](*slots[s])
                if SKEW <= s:
                    emitters[1](*slots[s - SKEW])


def round_f32r(x):
    """Round fp32 values to the fp32r subset (11 explicit mantissa bits,
    low 12 bits zero) with round-to-nearest-even — matches what the PE
    consumes in fp32r mode, so host-side rounding keeps hardware exact."""
    u = np.ascontiguousarray(x, dtype=np.float32).view(np.uint32)
    u = (u + 0x7FF + ((u >> 12) & 1)) & np.uint32(0xFFFFF000)
    return u.view(np.float32)


def make_in_maps(node, edge, time_steps, W1, W2, b):
    dts = np.asarray(time_steps, np.float32)
    dts = dts[1:] - dts[:-1]
    dt_vals = sorted({float(d) for d in dts})
    wcat = np.concatenate([W1, W2], axis=1).astype(np.float32)
    wcats = [wcat]
    w2s = [W2.astype(np.float32)]
    for dv in dt_vals:
        wcats.append(wcat * (dv / 2))
        wcats.append(wcat * dv)
        w2s.append(W2 * (dv / 2))
        w2s.append(W2 * dv)
    wcats = round_f32r(np.stack(wcats))
    w2s = round_f32r(np.stack(w2s))
    bc = np.ascontiguousarray(np.reshape(b, (D, 1)), dtype=np.float32)
    in_maps = []
    for core in range(NCORES):
        sl = slice(core * BPC, (core + 1) * BPC)
        in_maps.append(
            {
                "nodeT": round_f32r(node[sl].transpose(0, 2, 1)),
                "edgeT": round_f32r(edge[sl].transpose(0, 2, 1)),
                "wcats": wcats,
                "w2s": w2s,
                "bvec": bc,
            }
        )
    return in_maps


LAST_RESULT = None


def kernel(node, edge, time_steps, W1, W2, b, trace=False):
    node = np.asarray(node, dtype=np.float32)
    edge = np.asarray(edge, dtype=np.float32)
    time_steps = np.asarray(time_steps, dtype=np.float32)
    W1 = np.asarray(W1, dtype=np.float32)
    W2 = np.asarray(W2, dtype=np.float32)
    b = np.asarray(b, dtype=np.float32)

    dts = time_steps[1:] - time_steps[:-1]
    nc = build_program(dts)
    in_maps = make_in_maps(node, edge, time_steps, W1, W2, b)
    res = bass_utils.run_bass_kernel_spmd(
        nc, in_maps, core_ids=list(range(NCORES)), trace=trace
    )
    global LAST_RESULT
    LAST_RESULT = res
    outs = [res.results[c]["out"] for c in range(NCORES)]  # [T-1, BPC, D, N]
    full = np.concatenate(outs, axis=1)  # [T-1, B, D, N]
    pred = np.empty((T, B, N, D), dtype=np.float32)
    pred[0] = node
    pred[1:] = full.transpose(0, 1, 3, 2)
    return pred
